# revision 16
# baseline (speedup 1.0000x reference)
"""Multi-head attention (B=2, S=2048, H=8, Dh=32, D=256) on 8 TRN2 NeuronCores.

Sharding: core c -> (batch b = c//4, query-block qb = c%4 of 512 rows).
Each core computes full attention + output projection for its 512 query rows;
no cross-core communication is needed.  Host does layout prep only
(transposes + bf16 casts); all FLOPs run on device.

Device-side layout (per core):
  - raw activations shipped transposed: qT [256f, 512q], kT/vT [256f, 2048s]
  - q/k projections produced directly transposed (head-dim on partitions,
    head h at partitions 32*(h%4) of free-block h//4) so QK^T runs as
    K=32 row-tiled matmuls; the two heads of a pair are issued
    back-to-back at row positions 32r so the PE runs them concurrently.
  - scores computed TRANSPOSED: scoresT[k, q]; no max subtraction
    (scores ~ N(0,1), |s| < 10 measured, exp overflow at 88).
  - v projected to natural layout augmented with a ones column per head
    ([128, 16, 8, 33]) so each PV matmul (M=33) also accumulates the
    softmax denominator as its last output row - no separate reduction.
  - normalization: reciprocal of the two denominator rows (partitions 32
    and 96) + K=1 matmul row-broadcast + one full-tile DVE multiply.
  - final projection: K=32 matmuls per head slice against a host-permuted
    WoT whose row bands match the PV output partition bases.

Schedule (final): the 64 exp tiles ([128, 2, 512] PSUM score slots) are
split between TWO engines running concurrently (alternating slots):
  - ScalarE (ACT): exact exp ACTIVATE, ~1.07us/tile (34 tiles).
  - VectorE (DVE): Schraudolph bit-trick exp in ONE tensor_scalar op:
      int16(round(score * 128/ln2 + (16256 - 7.5))) bitcast to bf16
    ~= exp(score) with ~2% element error, ~1.21us/tile (30 tiles).
    HW-verified: f32->int16 conversion rounds to nearest and saturates.
    The softmax denominator is accumulated from the SAME approximated
    weights (ones column), so the ratio cancels most of the error;
    measured end-to-end rel err ~1.3e-2 vs the 2e-2 gate.
Slot order interleaves the two head pairs of each quad (slot = 32m +
2ct + t) so the paired ring-replacement fills use disjoint PE row bands
and PV work spreads evenly.  Aux elementwise work (projection casts,
denominator row copies, tail bias adds) is distributed between ACT and
DVE to balance their streams; ACT Copy/Identity share Exp's activation
table set so interleaving costs no table switches.

The PE's HAM clock gate is the sharpest constraint: the PE runs 2.4 GHz
only after ~3.4us of near-gapless activity (free-running 4096-cycle
windows) and falls to 1.2 GHz after any window with idle slivers.  A
26-dummy burst during the startup DMA wait earns the lease before real
work begins, and every slot pair is topped up with small dep-free dummy
matmuls ([128,256], ~150ns) toward a ~1.56us pair cadence; removing
them (or letting any region run dependency-gated without padding) was
measured to drop the clock for the rest of the run and cost 15-30%.
PSUM budget: 2x[128,2,512] rotating score slots + 2x[128,512] PV
accumulators + tmp + dummy bank = 8 banks.
"""

import sys

sys.path.insert(0, "/opt/trn_rl_repo")

import numpy as np
import ml_dtypes

import concourse.bass as bass
import concourse.bacc as bacc
import concourse.mybir as mybir
from concourse.tile import TileContext
from concourse.bass import ts
from concourse.bass_utils import run_bass_kernel_spmd

BF16 = mybir.dt.bfloat16
F32 = mybir.dt.float32
I16 = mybir.dt.int16
EXP = mybir.ActivationFunctionType.Exp
MULT = mybir.AluOpType.mult
ADD = mybir.AluOpType.add

B, SEQ, D = 2, 2048, 256
H, DH = 8, 32
QB = 512  # query rows per core
NKT = SEQ // 128  # 16 k-chunk tiles (partition tiles of scoresT)

# Schraudolph exp in bf16-bit space: bf16_bits(exp(s)) ~= s*K + B_SCH
SCH_K = float(128.0 / np.log(2.0))
SCH_B = 16256.0 - 7.5  # bias tuned on end-to-end softmax error

# exp-engine per slot j (64 slots): alternate ACT/DVE; a couple of extra
# ACT slots rebalance for DVE's aux-heavier stream.
_EXTRA_ACT = (33, 49)
ENG = ["A" if (j % 2 == 0 or j in _EXTRA_ACT) else "D" for j in range(64)]


def _build_graph():
    nc = bacc.Bacc("TRN2", target_bir_lowering=False, debug=False)

    wkq = nc.declare_dram_parameter("wkq", [D, 2 * D + QB], BF16, isOutput=False)
    kT = nc.declare_dram_parameter("kT", [D, SEQ], BF16, isOutput=False)
    vT = nc.declare_dram_parameter("vT", [D, SEQ], BF16, isOutput=False)
    wvT = nc.declare_dram_parameter("wvT", [D + 1, H * (DH + 1)], BF16, isOutput=False)
    woP = nc.declare_dram_parameter("woP", [128, 4 * D], BF16, isOutput=False)
    bo = nc.declare_dram_parameter("bo", [D, 1], F32, isOutput=False)
    outT = nc.declare_dram_parameter("outT", [D, QB], F32, isOutput=True)

    with TileContext(nc) as tc:
        with (
            tc.tile_pool(name="cst", bufs=1) as cst,
            tc.tile_pool(name="sb", bufs=1) as sb,
            tc.tile_pool(name="ps", bufs=2, space="PSUM") as ps,
        ):
            # ---- inputs.  All transfers ride the sync queue, with the
            # host-concatenated wk|wq|qT blob first (startup critical path).
            wkq_t = cst.tile([128, 2, 2 * D + QB], BF16)
            wk_t = wkq_t[:, :, 0:D]
            wq_t = wkq_t[:, :, D : 2 * D]
            qT_t = wkq_t[:, :, 2 * D :]
            kT_t = cst.tile([128, 2, SEQ], BF16)
            wv_t = cst.tile([128, 2, H * (DH + 1)], BF16)
            wva_t = cst.tile([1, H * (DH + 1)], BF16)
            vT_t = cst.tile([128, 2, SEQ], BF16)
            wo_t = cst.tile([128, 4, D], BF16)
            bo_t = cst.tile([128, 2, 1], F32)

            # ACT warm-up + exp table load before the stream starts.
            warm = cst.tile([1, 1], F32)
            nc.vector.memset(warm[:], 0.0)
            nc.scalar.activation(warm[:], warm[:], EXP)

            for f in range(2):
                nc.sync.dma_start(wkq_t[:, f, :], wkq[ts(f, 128), :])
            for f in range(2):
                nc.sync.dma_start(kT_t[:, f, ts(0, 512)], kT[ts(f, 128), ts(0, 512)])
            for f in range(2):
                nc.sync.dma_start(wv_t[:, f, :], wvT[ts(f, 128), :])
            nc.sync.dma_start(wva_t[:], wvT[D : D + 1, :])
            # early vT chunk: the interleaved schedule starts PV at slot ~6,
            # so the first v-projections need vT right after the k prefix.
            for f in range(2):
                nc.sync.dma_start(vT_t[:, f, ts(0, 512)], vT[ts(f, 128), ts(0, 512)])
            for f in range(2):
                nc.sync.dma_start(
                    kT_t[:, f, 512:1536], kT[ts(f, 128), 512:1536]
                )
            for f in range(2):
                nc.sync.dma_start(vT_t[:, f, 512:], vT[ts(f, 128), 512:])
            for f in range(2):
                nc.sync.dma_start(
                    kT_t[:, f, 1536:], kT[ts(f, 128), 1536:]
                )
            nc.sync.dma_start(wo_t[:, :, :], woP.rearrange("p (b d) -> p b d", b=4))
            for o in range(2):
                nc.sync.dma_start(bo_t[:, o, :], bo[ts(o, 128), :])

            # ---- SBUF destinations for projections.
            qp = cst.tile([128, 2, QB], BF16)  # q_projT
            kp = cst.tile([128, 2, SEQ], BF16)  # k_projT
            vp = cst.tile([128, NKT, H * (DH + 1)], BF16)  # v_proj + ones cols
            ones_t = cst.tile([128, 64], BF16)
            nc.vector.memset(ones_t[:], 1.0)
            vrow1 = cst.tile([1, SEQ], BF16)

            # ---- PE keep-warm machinery (HAM clock gate: the PE drops to
            # 1.2 GHz unless near-gaplessly busy; dep-free dummy matmuls
            # pepper the stream).  K=128 rows with an M=32 output keeps the
            # array visibly busy at a quarter of full PSUM write traffic.
            dmy = cst.tile([128, 256], BF16)
            nc.vector.memset(dmy[:], 0.0)
            nc.vector.memset(vrow1[:], 1.0)
            pdmy = ps.tile([128, 512], F32, tag="dmy", bufs=1, name="pdmy")

            def keep_warm(n):
                for _ in range(n):
                    nc.tensor.matmul(
                        pdmy[:32, 0:256],
                        ones_t[:, 0:32],
                        dmy[:, :],
                        start=True,
                        stop=True,
                    )

            # Earn the HAM warm lease during the startup DMA wait: the clock
            # only rises to 2.4 GHz after a sustained gapless stretch, so a
            # dense dummy burst here makes the real stream start warm.
            keep_warm(26)

            def copy_to(ce, dst, src):
                """PSUM f32 -> SBUF bf16 cast on the chosen engine."""
                if ce == "A":
                    nc.scalar.copy(dst, src)
                else:
                    nc.vector.tensor_copy(dst, src)

            # Transient projection PSUM pieces get a dedicated 1-slot ring
            # ("tmp"); "po" is reserved for the long-lived PV accumulators,
            # "sc" for the exp score slots.
            def proj_k(m, s4, ce):
                pk = ps.tile([128, 512], F32, tag="tmp", bufs=1, name=f"pk{m}{s4}")
                for f in range(2):
                    nc.tensor.matmul(
                        pk[:],
                        wk_t[:, f, ts(m, 128)],
                        kT_t[:, f, ts(s4, 512)],
                        start=(f == 0),
                        stop=(f == 1),
                    )
                copy_to(ce, kp[:, m, ts(s4, 512)], pk[:])

            def proj_q(m, ce):
                pq = ps.tile([128, QB], F32, tag="tmp", bufs=1, name=f"pq{m}")
                for f in range(2):
                    nc.tensor.matmul(
                        pq[:],
                        wq_t[:, f, ts(m, 128)],
                        qT_t[:, f, :],
                        start=(f == 0),
                        stop=(f == 1),
                    )
                copy_to(ce, qp[:, m, :], pq[:])

            def proj_v(st, ce):
                # third K=1 matmul of the host-side ones row against the
                # augmented Wv row produces the per-head ones columns, so
                # the PSUM->SBUF copy is fully contiguous.
                pv = ps.tile(
                    [128, H * (DH + 1)], F32, tag="tmp", bufs=1, name=f"pv{st}"
                )
                for f in range(2):
                    nc.tensor.matmul(
                        pv[:],
                        vT_t[:, f, ts(st, 128)],
                        wv_t[:, f, :],
                        start=(f == 0),
                        stop=False,
                    )
                nc.tensor.matmul(
                    pv[:],
                    vrow1[0:1, ts(st, 128)],
                    wva_t[:],
                    start=False,
                    stop=True,
                )
                copy_to(ce, vp[:, st, :], pv[:])

            # attn[(m, t)] = exp(scoresT) for head pair t of quad m:
            # [k-chunk part, ct, r', q]
            attn = {}
            for m in range(2):
                for t in range(2):
                    attn[(m, t)] = sb.tile(
                        [128, NKT, 2, 512], BF16, tag="attn", bufs=4,
                        name=f"attn{m}{t}",
                    )

            def qk_fill(m, t, ct):
                slot = ps.tile(
                    [128, 2, 512], F32, tag="sc", bufs=2, name=f"sc{m}{t}{ct}"
                )
                for rr in range(2):
                    r = 2 * t + rr
                    nc.tensor.matmul(
                        slot[:, rr, :],
                        kp[ts(r, 32), m, ts(ct, 128)],
                        qp[ts(r, 32), m, :],
                        start=True,
                        stop=True,
                        tile_position=(32 * r, 0),
                    )
                return slot

            def qk_act(m, t, ct, slot, eng):
                dst = attn[(m, t)][:, ct, :, :]
                if eng == "A":
                    nc.scalar.activation(dst, slot[:], EXP)
                else:
                    nc.vector.tensor_scalar(
                        dst.bitcast(I16), slot[:], SCH_K, SCH_B, MULT, ADD
                    )

            po = {}  # (m, t) -> PSUM accumulator [128, 512]

            def pv_pair(m, t, ct):
                """PV for both heads of pair (m,t), k-chunk ct.  The two
                matmuls sit at col positions 0/64 so they run concurrently."""
                if ct == 0:
                    po[(m, t)] = ps.tile(
                        [128, 512], F32, tag="po", bufs=2, name=f"po{m}{t}"
                    )
                p = po[(m, t)]
                for rr in range(2):
                    h = 4 * m + 2 * t + rr
                    base = 64 * rr
                    nc.tensor.matmul(
                        p[base : base + DH + 1, :],
                        vp[:, ct, ts(h, DH + 1)],
                        attn[(m, t)][:, ct, rr, :],
                        start=(ct == 0),
                        stop=(ct == NKT - 1),
                        tile_position=(0, base),
                        skip_group_check=True,
                    )

            prod = {}

            def stage_c(m, t, bc=None):
                """normalize: prod = po * (1 / PE-broadcast(denominator rows)).

                dsb copies ride ACT; reciprocal + the single full-tile
                multiply ride DVE (tensor_tensor is DVE-only)."""
                p = po[(m, t)]
                dsb = sb.tile([128, 512], BF16, tag="dsb", bufs=2, name=f"dsb{m}{t}")
                if bc is None:
                    bc = ps.tile([128, 512], F32, tag="tmp", bufs=1, name=f"bc{m}{t}")
                rsb = sb.tile([128, 512], F32, tag="rsb", bufs=2, name=f"rsb{m}{t}")
                prod[(m, t)] = sb.tile(
                    [128, 512], BF16, tag="prod", bufs=4, name=f"prod{m}{t}"
                )
                for base in (0, 64):
                    nc.scalar.copy(
                        dsb[base + DH : base + DH + 1, :],
                        p[base + DH : base + DH + 1, :],
                    )
                    # M=64 fills bc completely so the full-tile reciprocal
                    # below reads no stale slot bytes.
                    nc.tensor.matmul(
                        bc[base : base + 64, :],
                        ones_t[base + DH : base + DH + 1, :],
                        dsb[base + DH : base + DH + 1, :],
                        start=True,
                        stop=True,
                        tile_position=(base + DH, base),
                        skip_group_check=True,
                    )
                nc.vector.reciprocal_approx_fast(rsb[:], bc[:])
                # full-tile multiply: rows outside the head bands compute
                # garbage that nothing reads (pf matmuls take 0-31/64-95).
                nc.vector.tensor_mul(prod[(m, t)][:, :], p[:, :], rsb[:, :])

            # ================= schedule =================
            # startup: q-proj as soon as its (early) DMA lands, a dummy
            # batch to bridge the PE gap until kT arrives, then the k-proj
            # prefix for the first slots.  Both copies on DVE (idle then).
            proj_q(0, "D")
            keep_warm(4)
            proj_k(0, 0, "D")

            # work items carry a rough warm-PE cost (ns) so each slot gets
            # topped up with dummies to ~the exp drain cadence.
            def PV(m, t, ct):
                return (324, lambda: pv_pair(m, t, ct))

            def SC(m, t):
                return (546, lambda: stage_c(m, t))

            def PK(m, s4, ce):
                return (590, lambda: proj_k(m, s4, ce))

            def PQ(m, ce):
                return (590, lambda: proj_q(m, ce))

            def PVJ(st, ce):
                return (545, lambda: proj_v(st, ce))

            # Slot order INTERLEAVES the two head pairs of each quad:
            # slot(m, t, ct) = 32m + 2ct + t.  Consecutive slots are the
            # two pairs at the SAME k-chunk, so the two ring-replacement
            # fills issued together use disjoint PE row bands (rows
            # 0-63 for t=0, 64-127 for t=1) and stream 4-concurrently,
            # and PV work spreads evenly at 2 per slot pair instead of
            # bunching in the back phases.
            flat = []  # (m, t, ct)
            for m in range(2):
                for ct in range(NKT):
                    for t in range(2):
                        flat.append((m, t, ct))

            items = {j: [] for j in range(64)}
            # k/q projections: kT chunk s4 lands by ~slot 2-4; chunk ct
            # fills happen at slot 2ct so PK(m, s4) must complete ~8 slots
            # ahead of slot 8*s4 (m=0) / 32+8*s4 (m=1).
            items[0].append(PK(0, 1, "A"))
            items[4].append(PK(0, 2, "A"))
            items[8].append(PK(0, 3, "A"))
            items[12].append(PK(1, 0, "A"))
            items[14].append(PQ(1, "D"))
            items[16].append(PK(1, 1, "D"))
            items[20].append(PK(1, 2, "A"))
            items[24].append(PK(1, 3, "D"))
            # v projections: one per odd slot; vp[st] is consumed by PV at
            # slot 2st+6, giving the PSUM->SBUF copy 2 pairs of margin.
            _vce = ["D", "A"] * 8
            for st in range(NKT):
                items[2 * st + 1].append(PVJ(st, _vce[st]))
            # PV chases the acts with a 2-3 pair lag.  m=0 fits entirely
            # in-stream; m=1's last three chunks drain in the tail.
            for ct in range(NKT):
                for t in range(2):
                    items[2 * ct + 6 + t].append(PV(0, t, ct))
            for ct in range(13):
                for t in range(2):
                    items[38 + 2 * ct + t].append(PV(1, t, ct))
            # normalizations for quad 0 run right after its last PVs; the
            # SC(0,0) multiply must drain before PV(1,0,0) reuses its po
            # bank, so it leads slot 38's item list.
            items[38].insert(0, SC(0, 0))
            items[39].insert(1, SC(0, 1))

            slots = {}
            for j in range(2):
                slots[j] = qk_fill(*flat[j])
            for jp in range(32):
                j0, j1 = 2 * jp, 2 * jp + 1
                qk_act(*flat[j0], slots.pop(j0), ENG[j0])
                qk_act(*flat[j1], slots.pop(j1), ENG[j1])
                cost = 335  # fill pair
                for c, w in items[j0] + items[j1]:
                    w()
                    cost += c
                n_dmy = max(0, min(4, round((1560 - cost) / 215)))
                if jp < 8:
                    # the early pairs' items are DMA/copy-gated: their PE
                    # work stalls regardless of nominal cost, so keep a
                    # dummy floor to protect the clock lease.
                    n_dmy = max(n_dmy, 2)
                keep_warm(n_dmy)
                for j in (j0 + 2, j1 + 2):
                    if j < len(flat):
                        slots[j] = qk_fill(*flat[j])

            # ---- tail.  PSUM tiles in dependency-safe ring order: bc11
            # first (so the last normalize is never gated on the final
            # projection), then the final-projection accumulators.  3/4 of
            # the output projection runs before the last normalize; only
            # (m1,t1)'s K=32 contribution is serialized after it.
            # drain: the last three k-chunks of quad 1 (their acts occupy
            # the final stream slots), then the remaining normalizations.
            for ct in (13, 14, 15):
                for t in range(2):
                    pv_pair(1, t, ct)
            stage_c(1, 0)

            bc11 = ps.tile([128, 512], F32, tag="tmp", bufs=1, name="bc11")
            out_sb = cst.tile([128, 2, QB], F32)
            pf = {}
            pf[(0, 0)] = ps.tile([128, QB], F32, tag="sc", bufs=2, name="pf00")
            pf[(0, 64)] = ps.tile([128, QB], F32, tag="sc", bufs=2, name="pf064")
            pf[(1, 0)] = ps.tile([128, QB], F32, tag="po", bufs=2, name="pf10")
            pf[(1, 64)] = ps.tile([128, QB], F32, tag="dmy", bufs=1, name="pf164")

            def pf_mms(idx, m, t):
                for o in range(2):
                    for base in (0, 64):
                        nc.tensor.matmul(
                            pf[(o, base)][:],
                            wo_t[base : base + DH, 2 * m + t, ts(o, 128)],
                            prod[(m, t)][base : base + DH, :],
                            start=(idx == 0),
                            stop=(idx == 3),
                            tile_position=(base, 0),
                            skip_group_check=True,
                        )

            for idx, (m, t) in enumerate([(0, 0), (0, 1), (1, 0)]):
                pf_mms(idx, m, t)
            stage_c(1, 1, bc=bc11)
            pf_mms(3, 1, 1)
            for o in range(2):
                # bias add on ACT (Identity + per-partition bias), the
                # second accumulator add + out DMA on DVE/sync.
                nc.scalar.add(out_sb[:, o, :], pf[(o, 0)][:], bo_t[:, o, :])
                nc.vector.tensor_add(
                    out_sb[:, o, :], out_sb[:, o, :], pf[(o, 64)][:]
                )
                nc.sync.dma_start(outT[ts(o, 128), :], out_sb[:, o, :])

    nc.compile()
    return nc


_NC = None


def _get_nc():
    global _NC
    if _NC is None:
        _NC = _build_graph()
    return _NC


def prep_in_maps(query, key, value, Wq, Wk, Wv, Wo, bo):
    bf = ml_dtypes.bfloat16
    scale = np.float32(1.0 / np.sqrt(DH))

    wqT = np.ascontiguousarray((Wq.astype(np.float32) * scale).T).astype(bf)
    wkT = np.ascontiguousarray(Wk.T).astype(bf)
    # augmented WvT: [257 in-feats (last = ones row), 8 heads x 33 out-cols]
    wvT_a = np.zeros((D + 1, H * (DH + 1)), np.float32)
    wvt = Wv.T.astype(np.float32)  # [in 256, out 256]
    for h in range(H):
        wvT_a[:D, (DH + 1) * h : (DH + 1) * h + DH] = wvt[:, DH * h : DH * (h + 1)]
        wvT_a[D, (DH + 1) * h + DH] = 1.0
    wvT = np.ascontiguousarray(wvT_a).astype(bf)
    # permuted WoT: head h = 4m + 2t + rr lives at partition rows
    # 64*rr .. +32 of free-block 2m+t, matching PV output partitions.
    woP = np.zeros((128, 4, D), np.float32)
    woT = Wo.T.astype(np.float32)  # [hd, out]
    for h in range(H):
        m, r = h // 4, h % 4
        blk, base = 2 * m + r // 2, 64 * (r % 2)
        woP[base : base + DH, blk, :] = woT[DH * h : DH * (h + 1), :]
    woP = np.ascontiguousarray(woP.reshape(128, 4 * D)).astype(bf)
    bo_c = np.ascontiguousarray(bo.astype(np.float32).reshape(D, 1))

    kT_b = [np.ascontiguousarray(key[b].T).astype(bf) for b in range(B)]
    vT_b = [np.ascontiguousarray(value[b].T).astype(bf) for b in range(B)]

    in_maps = []
    for c in range(8):
        b, qb = c // 4, c % 4
        # one blob = wk | wq | qT so the startup-critical path is a single
        # DMA per f-half
        wkq = np.empty((D, 2 * D + QB), ml_dtypes.bfloat16)
        wkq[:, :D] = wkT
        wkq[:, D : 2 * D] = wqT
        wkq[:, 2 * D :] = query[b, qb * QB : (qb + 1) * QB, :].T.astype(bf)
        in_maps.append(
            {
                "wkq": np.ascontiguousarray(wkq),
                "kT": kT_b[b],
                "vT": vT_b[b],
                "wvT": wvT,
                "woP": woP,
                "bo": bo_c,
            }
        )
    return in_maps


def kernel(query, key, value, Wq, Wk, Wv, Wo, bo):
    nc = _get_nc()
    in_maps = prep_in_maps(query, key, value, Wq, Wk, Wv, Wo, bo)
    res = run_bass_kernel_spmd(nc, in_maps, core_ids=list(range(8)))

    out = np.empty((B, SEQ, D), np.float32)
    for c in range(8):
        b, qb = c // 4, c % 4
        out[b, qb * QB : (qb + 1) * QB, :] = res.results[c]["outT"].T
    return out


# revision 21
# speedup vs baseline: 1.0064x; 1.0064x over previous
"""Multi-head attention (B=2, S=2048, H=8, Dh=32, D=256) on 8 TRN2 NeuronCores.

Sharding: core c -> (batch b = c//4, query-block qb = c%4 of 512 rows).
Each core computes full attention + output projection for its 512 query rows;
no cross-core communication is needed.  Host does layout prep only
(transposes + bf16 casts); all FLOPs run on device.

Device-side layout (per core):
  - raw activations shipped transposed: qT [256f, 512q], kT/vT [256f, 2048s]
  - q/k projections produced directly transposed (head-dim on partitions,
    head h at partitions 32*(h%4) of free-block h//4) so QK^T runs as
    K=32 row-tiled matmuls; the two heads of a pair are issued
    back-to-back at row positions 32r so the PE runs them concurrently.
  - scores computed TRANSPOSED: scoresT[k, q]; no max subtraction
    (scores ~ N(0,1), |s| < 10 measured, exp overflow at 88).
  - v projected to natural layout augmented with a ones column per head
    ([128, 16, 8, 33]) so each PV matmul (M=33) also accumulates the
    softmax denominator as its last output row - no separate reduction.
  - normalization: reciprocal of the two denominator rows (partitions 32
    and 96) + K=1 matmul row-broadcast + one full-tile DVE multiply.
  - final projection: K=32 matmuls per head slice against a host-permuted
    WoT whose row bands match the PV output partition bases.

Schedule (final): the 64 exp tiles ([128, 2, 512] PSUM score slots) are
split between TWO engines running concurrently (alternating slots):
  - ScalarE (ACT): exact exp ACTIVATE, ~1.07us/tile (34 tiles).
  - VectorE (DVE): Schraudolph bit-trick exp in ONE tensor_scalar op:
      int16(round(score * 128/ln2 + (16256 - 7.5))) bitcast to bf16
    ~= exp(score) with ~2% element error, ~1.21us/tile (30 tiles).
    HW-verified: f32->int16 conversion rounds to nearest and saturates.
    The softmax denominator is accumulated from the SAME approximated
    weights (ones column), so the ratio cancels most of the error;
    measured end-to-end rel err ~1.3e-2 vs the 2e-2 gate.
Slot order interleaves the two head pairs of each quad (slot = 32m +
2ct + t) so the paired ring-replacement fills use disjoint PE row bands
and PV work spreads evenly.  Aux elementwise work (projection casts,
denominator row copies, tail bias adds) is distributed between ACT and
DVE to balance their streams; ACT Copy/Identity share Exp's activation
table set so interleaving costs no table switches.

The PE's HAM clock gate is the sharpest constraint: the PE runs 2.4 GHz
only after ~3.4us of near-gapless activity (free-running 4096-cycle
windows) and falls to 1.2 GHz after any window with idle slivers.  A
26-dummy burst during the startup DMA wait earns the lease before real
work begins, and every slot pair is topped up with small dep-free dummy
matmuls ([128,256], ~150ns) toward a ~1.56us pair cadence; removing
them (or letting any region run dependency-gated without padding) was
measured to drop the clock for the rest of the run and cost 15-30%.
PSUM budget: 3x[128,2,512] rotating score-ring buffers (the ring also
carries projection transients, dummy targets, and tail accumulators as
generations, so no dedicated tmp/dummy banks) + 2x[128,512] PV
accumulators = 8 banks.  The 3-deep ring relaxes each fill's WAR to the
act three generations back, taking the ring latency off the act chain.
"""

import sys

sys.path.insert(0, "/opt/trn_rl_repo")

import numpy as np
import ml_dtypes

import concourse.bass as bass
import concourse.bacc as bacc
import concourse.mybir as mybir
from concourse.tile import TileContext
from concourse.bass import ts
from concourse.bass_utils import run_bass_kernel_spmd

BF16 = mybir.dt.bfloat16
F32 = mybir.dt.float32
I16 = mybir.dt.int16
EXP = mybir.ActivationFunctionType.Exp
MULT = mybir.AluOpType.mult
ADD = mybir.AluOpType.add

B, SEQ, D = 2, 2048, 256
H, DH = 8, 32
QB = 512  # query rows per core
NKT = SEQ // 128  # 16 k-chunk tiles (partition tiles of scoresT)

# Schraudolph exp in bf16-bit space: bf16_bits(exp(s)) ~= s*K + B_SCH
SCH_K = float(128.0 / np.log(2.0))
SCH_B = 16256.0 - 7.5  # bias tuned on end-to-end softmax error

# exp-engine per slot j (64 slots): alternate ACT/DVE; a couple of extra
# ACT slots rebalance for DVE's aux-heavier stream.
_EXTRA_ACT = (33, 49)
ENG = ["A" if (j % 2 == 0 or j in _EXTRA_ACT) else "D" for j in range(64)]


def _build_graph():
    nc = bacc.Bacc("TRN2", target_bir_lowering=False, debug=False)

    wkq = nc.declare_dram_parameter("wkq", [D, 2 * D + QB], BF16, isOutput=False)
    kT = nc.declare_dram_parameter("kT", [D, SEQ], BF16, isOutput=False)
    vT = nc.declare_dram_parameter("vT", [D, SEQ], BF16, isOutput=False)
    wvT = nc.declare_dram_parameter("wvT", [D + 1, H * (DH + 1)], BF16, isOutput=False)
    woP = nc.declare_dram_parameter("woP", [128, 4 * D], BF16, isOutput=False)
    bo = nc.declare_dram_parameter("bo", [D, 1], F32, isOutput=False)
    outT = nc.declare_dram_parameter("outT", [D, QB], F32, isOutput=True)

    with TileContext(nc) as tc:
        with (
            tc.tile_pool(name="cst", bufs=1) as cst,
            tc.tile_pool(name="sb", bufs=1) as sb,
            tc.tile_pool(name="ps", bufs=2, space="PSUM") as ps,
        ):
            # ---- inputs.  All transfers ride the sync queue, with the
            # host-concatenated wk|wq|qT blob first (startup critical path).
            wkq_t = cst.tile([128, 2, 2 * D + QB], BF16)
            wk_t = wkq_t[:, :, 0:D]
            wq_t = wkq_t[:, :, D : 2 * D]
            qT_t = wkq_t[:, :, 2 * D :]
            kT_t = cst.tile([128, 2, SEQ], BF16)
            wv_t = cst.tile([128, 2, H * (DH + 1)], BF16)
            wva_t = cst.tile([1, H * (DH + 1)], BF16)
            vT_t = cst.tile([128, 2, SEQ], BF16)
            wo_t = cst.tile([128, 4, D], BF16)
            bo_t = cst.tile([128, 2, 1], F32)

            # ACT warm-up + exp table load before the stream starts.
            warm = cst.tile([1, 1], F32)
            nc.vector.memset(warm[:], 0.0)
            nc.scalar.activation(warm[:], warm[:], EXP)

            for f in range(2):
                nc.sync.dma_start(wkq_t[:, f, :], wkq[ts(f, 128), :])
            for f in range(2):
                nc.sync.dma_start(kT_t[:, f, ts(0, 512)], kT[ts(f, 128), ts(0, 512)])
            for f in range(2):
                nc.sync.dma_start(wv_t[:, f, :], wvT[ts(f, 128), :])
            nc.sync.dma_start(wva_t[:], wvT[D : D + 1, :])
            # early vT chunk: the interleaved schedule starts PV at slot ~6,
            # so the first v-projections need vT right after the k prefix.
            for f in range(2):
                nc.sync.dma_start(vT_t[:, f, ts(0, 512)], vT[ts(f, 128), ts(0, 512)])
            for f in range(2):
                nc.sync.dma_start(
                    kT_t[:, f, 512:1536], kT[ts(f, 128), 512:1536]
                )
            for f in range(2):
                nc.sync.dma_start(vT_t[:, f, 512:], vT[ts(f, 128), 512:])
            for f in range(2):
                nc.sync.dma_start(
                    kT_t[:, f, 1536:], kT[ts(f, 128), 1536:]
                )
            nc.sync.dma_start(wo_t[:, :, :], woP.rearrange("p (b d) -> p b d", b=4))
            for o in range(2):
                nc.sync.dma_start(bo_t[:, o, :], bo[ts(o, 128), :])

            # ---- SBUF destinations for projections.
            qp = cst.tile([128, 2, QB], BF16)  # q_projT
            kp = cst.tile([128, 2, SEQ], BF16)  # k_projT
            vp = cst.tile([128, NKT, H * (DH + 1)], BF16)  # v_proj + ones cols
            ones_t = cst.tile([128, 64], BF16)
            nc.vector.memset(ones_t[:], 1.0)
            vrow1 = cst.tile([1, SEQ], BF16)

            # ---- PE keep-warm machinery (HAM clock gate: the PE drops to
            # 1.2 GHz unless near-gaplessly busy; dep-free dummy matmuls
            # pepper the stream).  K=128 rows with an M=32 output keeps the
            # array visibly busy at a quarter of full PSUM write traffic.
            dmy = cst.tile([128, 256], BF16)
            nc.vector.memset(dmy[:], 0.0)
            nc.vector.memset(vrow1[:], 1.0)

            # Dummies write into a fresh score-ring generation each call:
            # with the 3-deep ring this costs no dedicated PSUM bank, and
            # the WAR against the act 3 generations back is already
            # satisfied whenever the ring is ahead (exactly the situations
            # where the PE needs padding).
            _kw = [0]

            def keep_warm(n):
                if n <= 0:
                    return
                _kw[0] += 1
                pd = ps.tile(
                    [128, 512], F32, tag="sc", bufs=3, name=f"pd{_kw[0]}"
                )
                for _ in range(n):
                    nc.tensor.matmul(
                        pd[:32, 0:256],
                        ones_t[:, 0:32],
                        dmy[:, :],
                        start=True,
                        stop=True,
                    )

            # Earn the HAM warm lease during the startup DMA wait: the clock
            # only rises to 2.4 GHz after a sustained gapless stretch, so a
            # dense dummy burst here makes the real stream start warm.
            keep_warm(26)

            def copy_to(ce, dst, src):
                """PSUM f32 -> SBUF bf16 cast on the chosen engine."""
                if ce == "A":
                    nc.scalar.copy(dst, src)
                else:
                    nc.vector.tensor_copy(dst, src)

            # Transient projection PSUM pieces get a dedicated 1-slot ring
            # ("tmp"); "po" is reserved for the long-lived PV accumulators,
            # "sc" for the exp score slots.
            def proj_k(m, s4, ce):
                pk = ps.tile([128, 512], F32, tag="sc", bufs=3, name=f"pk{m}{s4}")
                for f in range(2):
                    nc.tensor.matmul(
                        pk[:],
                        wk_t[:, f, ts(m, 128)],
                        kT_t[:, f, ts(s4, 512)],
                        start=(f == 0),
                        stop=(f == 1),
                    )
                copy_to(ce, kp[:, m, ts(s4, 512)], pk[:])

            def proj_q(m, ce):
                pq = ps.tile([128, QB], F32, tag="sc", bufs=3, name=f"pq{m}")
                for f in range(2):
                    nc.tensor.matmul(
                        pq[:],
                        wq_t[:, f, ts(m, 128)],
                        qT_t[:, f, :],
                        start=(f == 0),
                        stop=(f == 1),
                    )
                copy_to(ce, qp[:, m, :], pq[:])

            def proj_v(st, ce):
                # third K=1 matmul of the host-side ones row against the
                # augmented Wv row produces the per-head ones columns, so
                # the PSUM->SBUF copy is fully contiguous.
                pv = ps.tile(
                    [128, H * (DH + 1)], F32, tag="sc", bufs=3, name=f"pv{st}"
                )
                for f in range(2):
                    nc.tensor.matmul(
                        pv[:],
                        vT_t[:, f, ts(st, 128)],
                        wv_t[:, f, :],
                        start=(f == 0),
                        stop=False,
                    )
                nc.tensor.matmul(
                    pv[:],
                    vrow1[0:1, ts(st, 128)],
                    wva_t[:],
                    start=False,
                    stop=True,
                )
                copy_to(ce, vp[:, st, :], pv[:])

            # attn[(m, t)] = exp(scoresT) for head pair t of quad m:
            # [k-chunk part, ct, r', q]
            attn = {}
            for m in range(2):
                for t in range(2):
                    attn[(m, t)] = sb.tile(
                        [128, NKT, 2, 512], BF16, tag="attn", bufs=4,
                        name=f"attn{m}{t}",
                    )

            def qk_fill(m, t, ct):
                slot = ps.tile(
                    [128, 2, 512], F32, tag="sc", bufs=3, name=f"sc{m}{t}{ct}"
                )
                for rr in range(2):
                    r = 2 * t + rr
                    nc.tensor.matmul(
                        slot[:, rr, :],
                        kp[ts(r, 32), m, ts(ct, 128)],
                        qp[ts(r, 32), m, :],
                        start=True,
                        stop=True,
                        tile_position=(32 * r, 0),
                    )
                return slot

            def qk_act(m, t, ct, slot, eng):
                dst = attn[(m, t)][:, ct, :, :]
                if eng == "A":
                    nc.scalar.activation(dst, slot[:], EXP)
                else:
                    nc.vector.tensor_scalar(
                        dst.bitcast(I16), slot[:], SCH_K, SCH_B, MULT, ADD
                    )

            po = {}  # (m, t) -> PSUM accumulator [128, 512]

            def pv_pair(m, t, ct):
                """PV for both heads of pair (m,t), k-chunk ct.  The two
                matmuls sit at col positions 0/64 so they run concurrently."""
                if ct == 0:
                    po[(m, t)] = ps.tile(
                        [128, 512], F32, tag="po", bufs=2, name=f"po{m}{t}"
                    )
                p = po[(m, t)]
                for rr in range(2):
                    h = 4 * m + 2 * t + rr
                    base = 64 * rr
                    nc.tensor.matmul(
                        p[base : base + DH + 1, :],
                        vp[:, ct, ts(h, DH + 1)],
                        attn[(m, t)][:, ct, rr, :],
                        start=(ct == 0),
                        stop=(ct == NKT - 1),
                        tile_position=(0, base),
                        skip_group_check=True,
                    )

            prod = {}

            def stage_c(m, t, bc=None):
                """normalize: prod = po * (1 / PE-broadcast(denominator rows)).

                dsb copies ride ACT; reciprocal + the single full-tile
                multiply ride DVE (tensor_tensor is DVE-only)."""
                p = po[(m, t)]
                dsb = sb.tile([128, 512], BF16, tag="dsb", bufs=2, name=f"dsb{m}{t}")
                if bc is None:
                    bc = ps.tile([128, 512], F32, tag="sc", bufs=3, name=f"bc{m}{t}")
                rsb = sb.tile([128, 512], F32, tag="rsb", bufs=2, name=f"rsb{m}{t}")
                prod[(m, t)] = sb.tile(
                    [128, 512], BF16, tag="prod", bufs=4, name=f"prod{m}{t}"
                )
                for base in (0, 64):
                    nc.scalar.copy(
                        dsb[base + DH : base + DH + 1, :],
                        p[base + DH : base + DH + 1, :],
                    )
                    # M=64 fills bc completely so the full-tile reciprocal
                    # below reads no stale slot bytes.
                    nc.tensor.matmul(
                        bc[base : base + 64, :],
                        ones_t[base + DH : base + DH + 1, :],
                        dsb[base + DH : base + DH + 1, :],
                        start=True,
                        stop=True,
                        tile_position=(base + DH, base),
                        skip_group_check=True,
                    )
                nc.vector.reciprocal_approx_fast(rsb[:], bc[:])
                # full-tile multiply: rows outside the head bands compute
                # garbage that nothing reads (pf matmuls take 0-31/64-95).
                nc.vector.tensor_mul(prod[(m, t)][:, :], p[:, :], rsb[:, :])

            # ================= schedule =================
            # startup: q-proj as soon as its (early) DMA lands, a dummy
            # batch to bridge the PE gap until kT arrives, then the k-proj
            # prefix for the first slots.  Both copies on DVE (idle then).
            proj_q(0, "D")
            keep_warm(4)
            proj_k(0, 0, "D")

            # work items carry a rough warm-PE cost (ns) so each slot gets
            # topped up with dummies to ~the exp drain cadence.
            def PV(m, t, ct):
                return (324, lambda: pv_pair(m, t, ct))

            def SC(m, t):
                return (546, lambda: stage_c(m, t))

            def PK(m, s4, ce):
                return (590, lambda: proj_k(m, s4, ce))

            def PQ(m, ce):
                return (590, lambda: proj_q(m, ce))

            def PVJ(st, ce):
                return (545, lambda: proj_v(st, ce))

            # Slot order INTERLEAVES the two head pairs of each quad:
            # slot(m, t, ct) = 32m + 2ct + t.  Consecutive slots are the
            # two pairs at the SAME k-chunk, so the two ring-replacement
            # fills issued together use disjoint PE row bands (rows
            # 0-63 for t=0, 64-127 for t=1) and stream 4-concurrently,
            # and PV work spreads evenly at 2 per slot pair instead of
            # bunching in the back phases.
            flat = []  # (m, t, ct)
            for m in range(2):
                for ct in range(NKT):
                    for t in range(2):
                        flat.append((m, t, ct))

            items = {j: [] for j in range(64)}
            # k/q projections: kT chunk s4 lands by ~slot 2-4; chunk ct
            # fills happen at slot 2ct so PK(m, s4) must complete ~8 slots
            # ahead of slot 8*s4 (m=0) / 32+8*s4 (m=1).
            items[0].append(PK(0, 1, "A"))
            items[4].append(PK(0, 2, "A"))
            items[8].append(PK(0, 3, "A"))
            items[12].append(PK(1, 0, "A"))
            items[14].append(PQ(1, "D"))
            items[16].append(PK(1, 1, "D"))
            items[20].append(PK(1, 2, "A"))
            items[24].append(PK(1, 3, "D"))
            # v projections: one per odd slot; vp[st] is consumed by PV at
            # slot 2st+6, giving the PSUM->SBUF copy 2 pairs of margin.
            _vce = ["D", "A"] * 8
            for st in range(NKT):
                items[2 * st + 1].append(PVJ(st, _vce[st]))
            # PV chases the acts with a 2-3 pair lag.  m=0 fits entirely
            # in-stream; m=1's last three chunks drain in the tail.
            for ct in range(NKT):
                for t in range(2):
                    items[2 * ct + 6 + t].append(PV(0, t, ct))
            for ct in range(13):
                for t in range(2):
                    items[38 + 2 * ct + t].append(PV(1, t, ct))
            # normalizations for quad 0 run right after its last PVs; the
            # SC(0,0) multiply must drain before PV(1,0,0) reuses its po
            # bank, so it leads slot 38's item list.
            items[38].insert(0, SC(0, 0))
            items[39].insert(1, SC(0, 1))

            slots = {}
            for j in range(2):
                slots[j] = qk_fill(*flat[j])
            for jp in range(32):
                j0, j1 = 2 * jp, 2 * jp + 1
                qk_act(*flat[j0], slots.pop(j0), ENG[j0])
                qk_act(*flat[j1], slots.pop(j1), ENG[j1])
                cost = 335  # fill pair
                for c, w in items[j0] + items[j1]:
                    w()
                    cost += c
                n_dmy = max(0, min(4, round((1560 - cost) / 215)))
                if jp < 8:
                    # the early pairs' items are DMA/copy-gated: their PE
                    # work stalls regardless of nominal cost, so keep a
                    # dummy floor to protect the clock lease.
                    n_dmy = max(n_dmy, 2)
                keep_warm(n_dmy)
                for j in (j0 + 2, j1 + 2):
                    if j < len(flat):
                        slots[j] = qk_fill(*flat[j])

            # ---- tail.  PSUM tiles in dependency-safe ring order: bc11
            # first (so the last normalize is never gated on the final
            # projection), then the final-projection accumulators.  3/4 of
            # the output projection runs before the last normalize; only
            # (m1,t1)'s K=32 contribution is serialized after it.
            # drain: the last three k-chunks of quad 1 (their acts occupy
            # the final stream slots), then BOTH remaining normalizations.
            # The pf accumulators are allocated only after both stage_c's
            # so every ring-buffer reuse WARs an already-issued reader
            # (allocating them earlier deadlocks: the pf writes would be
            # ordered before the bc broadcast they transitively feed).
            for ct in (13, 14, 15):
                for t in range(2):
                    pv_pair(1, t, ct)
            stage_c(1, 0)
            stage_c(1, 1)

            out_sb = cst.tile([128, 2, QB], F32)
            pf = {}
            pf[(0, 0)] = ps.tile([128, QB], F32, tag="sc", bufs=3, name="pf00")
            pf[(0, 64)] = ps.tile([128, QB], F32, tag="sc", bufs=3, name="pf064")
            pf[(1, 0)] = ps.tile([128, QB], F32, tag="po", bufs=2, name="pf10")
            pf[(1, 64)] = ps.tile([128, QB], F32, tag="po", bufs=2, name="pf164")

            def pf_mms(idx, m, t):
                for o in range(2):
                    for base in (0, 64):
                        nc.tensor.matmul(
                            pf[(o, base)][:],
                            wo_t[base : base + DH, 2 * m + t, ts(o, 128)],
                            prod[(m, t)][base : base + DH, :],
                            start=(idx == 0),
                            stop=(idx == 3),
                            tile_position=(base, 0),
                            skip_group_check=True,
                        )

            for idx, (m, t) in enumerate([(0, 0), (0, 1), (1, 0), (1, 1)]):
                pf_mms(idx, m, t)
            for o in range(2):
                # bias add on ACT (Identity + per-partition bias), the
                # second accumulator add + out DMA on DVE/sync.
                nc.scalar.add(out_sb[:, o, :], pf[(o, 0)][:], bo_t[:, o, :])
                nc.vector.tensor_add(
                    out_sb[:, o, :], out_sb[:, o, :], pf[(o, 64)][:]
                )
                nc.sync.dma_start(outT[ts(o, 128), :], out_sb[:, o, :])

    nc.compile()
    return nc


_NC = None


def _get_nc():
    global _NC
    if _NC is None:
        _NC = _build_graph()
    return _NC


def prep_in_maps(query, key, value, Wq, Wk, Wv, Wo, bo):
    bf = ml_dtypes.bfloat16
    scale = np.float32(1.0 / np.sqrt(DH))

    wqT = np.ascontiguousarray((Wq.astype(np.float32) * scale).T).astype(bf)
    wkT = np.ascontiguousarray(Wk.T).astype(bf)
    # augmented WvT: [257 in-feats (last = ones row), 8 heads x 33 out-cols]
    wvT_a = np.zeros((D + 1, H * (DH + 1)), np.float32)
    wvt = Wv.T.astype(np.float32)  # [in 256, out 256]
    for h in range(H):
        wvT_a[:D, (DH + 1) * h : (DH + 1) * h + DH] = wvt[:, DH * h : DH * (h + 1)]
        wvT_a[D, (DH + 1) * h + DH] = 1.0
    wvT = np.ascontiguousarray(wvT_a).astype(bf)
    # permuted WoT: head h = 4m + 2t + rr lives at partition rows
    # 64*rr .. +32 of free-block 2m+t, matching PV output partitions.
    woP = np.zeros((128, 4, D), np.float32)
    woT = Wo.T.astype(np.float32)  # [hd, out]
    for h in range(H):
        m, r = h // 4, h % 4
        blk, base = 2 * m + r // 2, 64 * (r % 2)
        woP[base : base + DH, blk, :] = woT[DH * h : DH * (h + 1), :]
    woP = np.ascontiguousarray(woP.reshape(128, 4 * D)).astype(bf)
    bo_c = np.ascontiguousarray(bo.astype(np.float32).reshape(D, 1))

    kT_b = [np.ascontiguousarray(key[b].T).astype(bf) for b in range(B)]
    vT_b = [np.ascontiguousarray(value[b].T).astype(bf) for b in range(B)]

    in_maps = []
    for c in range(8):
        b, qb = c // 4, c % 4
        # one blob = wk | wq | qT so the startup-critical path is a single
        # DMA per f-half
        wkq = np.empty((D, 2 * D + QB), ml_dtypes.bfloat16)
        wkq[:, :D] = wkT
        wkq[:, D : 2 * D] = wqT
        wkq[:, 2 * D :] = query[b, qb * QB : (qb + 1) * QB, :].T.astype(bf)
        in_maps.append(
            {
                "wkq": np.ascontiguousarray(wkq),
                "kT": kT_b[b],
                "vT": vT_b[b],
                "wvT": wvT,
                "woP": woP,
                "bo": bo_c,
            }
        )
    return in_maps


def kernel(query, key, value, Wq, Wk, Wv, Wo, bo):
    nc = _get_nc()
    in_maps = prep_in_maps(query, key, value, Wq, Wk, Wv, Wo, bo)
    res = run_bass_kernel_spmd(nc, in_maps, core_ids=list(range(8)))

    out = np.empty((B, SEQ, D), np.float32)
    for c in range(8):
        b, qb = c // 4, c % 4
        out[b, qb * QB : (qb + 1) * QB, :] = res.results[c]["outT"].T
    return out


# revision 24
# speedup vs baseline: 1.0143x; 1.0078x over previous
"""Multi-head attention (B=2, S=2048, H=8, Dh=32, D=256) on 8 TRN2 NeuronCores.

Sharding: core c -> (batch b = c//4, query-block qb = c%4 of 512 rows).
Each core computes full attention + output projection for its 512 query rows;
no cross-core communication is needed.  Host does layout prep only
(transposes + bf16 casts); all FLOPs run on device.

Device-side layout (per core):
  - raw activations shipped transposed: qT [256f, 512q], kT/vT [256f, 2048s]
  - q/k projections produced directly transposed (head-dim on partitions,
    head h at partitions 32*(h%4) of free-block h//4) so QK^T runs as
    K=32 row-tiled matmuls; the two heads of a pair are issued
    back-to-back at row positions 32r so the PE runs them concurrently.
  - scores computed TRANSPOSED: scoresT[k, q]; no max subtraction
    (scores ~ N(0,1), |s| < 10 measured, exp overflow at 88).
  - v projected to natural layout augmented with a ones column per head
    ([128, 16, 8, 33]) so each PV matmul (M=33) also accumulates the
    softmax denominator as its last output row - no separate reduction.
  - normalization: reciprocal of the two denominator rows (partitions 32
    and 96) + K=1 matmul row-broadcast + one full-tile DVE multiply.
  - final projection: K=32 matmuls per head slice against a host-permuted
    WoT whose row bands match the PV output partition bases.

Schedule (final): the 64 exp tiles ([128, 2, 512] PSUM score slots) are
split between TWO engines running concurrently (alternating slots):
  - ScalarE (ACT): exact exp ACTIVATE, ~1.07us/tile (34 tiles).
  - VectorE (DVE): Schraudolph bit-trick exp in ONE tensor_scalar op:
      int16(round(score * 128/ln2 + (16256 - 7.5))) bitcast to bf16
    ~= exp(score) with ~2% element error, ~1.21us/tile (30 tiles).
    HW-verified: f32->int16 conversion rounds to nearest and saturates.
    The softmax denominator is accumulated from the SAME approximated
    weights (ones column), so the ratio cancels most of the error;
    measured end-to-end rel err ~1.3e-2 vs the 2e-2 gate.
Slot order interleaves the two head pairs of each quad (slot = 32m +
2ct + t) so the paired ring-replacement fills use disjoint PE row bands
and PV work spreads evenly.  Aux elementwise work (projection casts,
denominator row copies, tail bias adds) is distributed between ACT and
DVE to balance their streams; ACT Copy/Identity share Exp's activation
table set so interleaving costs no table switches.

The PE's HAM clock gate is the sharpest constraint: the PE runs 2.4 GHz
only after ~3.4us of near-gapless activity (free-running 4096-cycle
windows) and falls to 1.2 GHz after any window with idle slivers.  A
26-dummy burst during the startup DMA wait earns the lease before real
work begins, and every slot pair is topped up with small dep-free dummy
matmuls ([128,256], ~150ns) toward a ~1.56us pair cadence; removing
them (or letting any region run dependency-gated without padding) was
measured to drop the clock for the rest of the run and cost 15-30%.
PSUM budget: 3x[128,2,512] rotating score-ring buffers (the ring also
carries projection transients, dummy targets, and tail accumulators as
generations, so no dedicated tmp/dummy banks) + 2x[128,512] PV
accumulators = 8 banks.  The 3-deep ring relaxes each fill's WAR to the
act three generations back, taking the ring latency off the act chain.
"""

import sys

sys.path.insert(0, "/opt/trn_rl_repo")

import numpy as np
import ml_dtypes

import concourse.bass as bass
import concourse.bacc as bacc
import concourse.mybir as mybir
from concourse.tile import TileContext
from concourse.bass import ts
from concourse.bass_utils import run_bass_kernel_spmd

BF16 = mybir.dt.bfloat16
F32 = mybir.dt.float32
I16 = mybir.dt.int16
EXP = mybir.ActivationFunctionType.Exp
MULT = mybir.AluOpType.mult
ADD = mybir.AluOpType.add

B, SEQ, D = 2, 2048, 256
H, DH = 8, 32
QB = 512  # query rows per core
NKT = SEQ // 128  # 16 k-chunk tiles (partition tiles of scoresT)

# Schraudolph exp in bf16-bit space: bf16_bits(exp(s)) ~= s*K + B_SCH
SCH_K = float(128.0 / np.log(2.0))
SCH_B = 16256.0 - 7.5  # bias tuned on end-to-end softmax error

# exp-engine per slot j (64 slots): alternate ACT/DVE; a couple of extra
# ACT slots rebalance for DVE's aux-heavier stream.
_EXTRA_ACT = (33, 49)
ENG = ["A" if (j % 2 == 0 or j in _EXTRA_ACT) else "D" for j in range(64)]


def _build_graph():
    nc = bacc.Bacc("TRN2", target_bir_lowering=False, debug=False)

    wkq = nc.declare_dram_parameter("wkq", [D, 2 * D + QB], BF16, isOutput=False)
    kT = nc.declare_dram_parameter("kT", [D, SEQ], BF16, isOutput=False)
    vT = nc.declare_dram_parameter("vT", [D, SEQ], BF16, isOutput=False)
    wvT = nc.declare_dram_parameter("wvT", [D + 1, H * (DH + 1)], BF16, isOutput=False)
    woP = nc.declare_dram_parameter("woP", [128, 4 * D], BF16, isOutput=False)
    bo = nc.declare_dram_parameter("bo", [D, 1], F32, isOutput=False)
    outT = nc.declare_dram_parameter("outT", [D, QB], F32, isOutput=True)

    with TileContext(nc) as tc:
        with (
            tc.tile_pool(name="cst", bufs=1) as cst,
            tc.tile_pool(name="sb", bufs=1) as sb,
            tc.tile_pool(name="ps", bufs=2, space="PSUM") as ps,
        ):
            # ---- inputs.  All transfers ride the sync queue, with the
            # host-concatenated wk|wq|qT blob first (startup critical path).
            wkq_t = cst.tile([128, 2, 2 * D + QB], BF16)
            wk_t = wkq_t[:, :, 0:D]
            wq_t = wkq_t[:, :, D : 2 * D]
            qT_t = wkq_t[:, :, 2 * D :]
            kT_t = cst.tile([128, 2, SEQ], BF16)
            wv_t = cst.tile([128, 2, H * (DH + 1)], BF16)
            wva_t = cst.tile([1, H * (DH + 1)], BF16)
            vT_t = cst.tile([128, 2, SEQ], BF16)
            wo_t = cst.tile([128, 4, D], BF16)
            bo_t = cst.tile([128, 2, 1], F32)

            # ACT warm-up + exp table load before the stream starts.
            warm = cst.tile([1, 1], F32)
            nc.vector.memset(warm[:], 0.0)
            nc.scalar.activation(warm[:], warm[:], EXP)

            for f in range(2):
                nc.sync.dma_start(wkq_t[:, f, :], wkq[ts(f, 128), :])
            for f in range(2):
                nc.sync.dma_start(kT_t[:, f, ts(0, 512)], kT[ts(f, 128), ts(0, 512)])
            for f in range(2):
                nc.sync.dma_start(wv_t[:, f, :], wvT[ts(f, 128), :])
            nc.sync.dma_start(wva_t[:], wvT[D : D + 1, :])
            # early vT chunk: the interleaved schedule starts PV at slot ~6,
            # so the first v-projections need vT right after the k prefix.
            for f in range(2):
                nc.sync.dma_start(vT_t[:, f, ts(0, 512)], vT[ts(f, 128), ts(0, 512)])
            for f in range(2):
                nc.sync.dma_start(
                    kT_t[:, f, 512:1536], kT[ts(f, 128), 512:1536]
                )
            for f in range(2):
                nc.sync.dma_start(vT_t[:, f, 512:], vT[ts(f, 128), 512:])
            for f in range(2):
                nc.sync.dma_start(
                    kT_t[:, f, 1536:], kT[ts(f, 128), 1536:]
                )
            nc.sync.dma_start(wo_t[:, :, :], woP.rearrange("p (b d) -> p b d", b=4))
            for o in range(2):
                nc.sync.dma_start(bo_t[:, o, :], bo[ts(o, 128), :])

            # ---- SBUF destinations for projections.
            qp = cst.tile([128, 2, QB], BF16)  # q_projT
            kp = cst.tile([128, 2, SEQ], BF16)  # k_projT
            vp = cst.tile([128, NKT, H * (DH + 1)], BF16)  # v_proj + ones cols
            ones_t = cst.tile([128, 64], BF16)
            nc.vector.memset(ones_t[:], 1.0)
            vrow1 = cst.tile([1, SEQ], BF16)

            # ---- PE keep-warm machinery (HAM clock gate: the PE drops to
            # 1.2 GHz unless near-gaplessly busy; dep-free dummy matmuls
            # pepper the stream).  K=128 rows with an M=32 output keeps the
            # array visibly busy at a quarter of full PSUM write traffic.
            dmy = cst.tile([128, 256], BF16)
            nc.vector.memset(dmy[:], 0.0)
            nc.vector.memset(vrow1[:], 1.0)

            # Dummies write into a fresh score-ring generation each call:
            # with the 3-deep ring this costs no dedicated PSUM bank, and
            # the WAR against the act 3 generations back is already
            # satisfied whenever the ring is ahead (exactly the situations
            # where the PE needs padding).
            _kw = [0]

            def keep_warm(n):
                if n <= 0:
                    return
                _kw[0] += 1
                pd = ps.tile(
                    [128, 512], F32, tag="sc", bufs=3, name=f"pd{_kw[0]}"
                )
                for _ in range(n):
                    nc.tensor.matmul(
                        pd[:32, 0:256],
                        ones_t[:, 0:32],
                        dmy[:, :],
                        start=True,
                        stop=True,
                    )

            # Earn the HAM warm lease during the startup DMA wait: the clock
            # only rises to 2.4 GHz after a sustained gapless stretch, so a
            # dense dummy burst here makes the real stream start warm.
            keep_warm(30)

            def copy_to(ce, dst, src):
                """PSUM f32 -> SBUF bf16 cast on the chosen engine."""
                if ce == "A":
                    nc.scalar.copy(dst, src)
                else:
                    nc.vector.tensor_copy(dst, src)

            # Transient projection PSUM pieces get a dedicated 1-slot ring
            # ("tmp"); "po" is reserved for the long-lived PV accumulators,
            # "sc" for the exp score slots.
            def proj_k(m, s4, ce):
                pk = ps.tile([128, 512], F32, tag="sc", bufs=3, name=f"pk{m}{s4}")
                for f in range(2):
                    nc.tensor.matmul(
                        pk[:],
                        wk_t[:, f, ts(m, 128)],
                        kT_t[:, f, ts(s4, 512)],
                        start=(f == 0),
                        stop=(f == 1),
                    )
                copy_to(ce, kp[:, m, ts(s4, 512)], pk[:])

            def proj_q(m, ce):
                pq = ps.tile([128, QB], F32, tag="sc", bufs=3, name=f"pq{m}")
                for f in range(2):
                    nc.tensor.matmul(
                        pq[:],
                        wq_t[:, f, ts(m, 128)],
                        qT_t[:, f, :],
                        start=(f == 0),
                        stop=(f == 1),
                    )
                copy_to(ce, qp[:, m, :], pq[:])

            def proj_v(st, ce):
                # third K=1 matmul of the host-side ones row against the
                # augmented Wv row produces the per-head ones columns, so
                # the PSUM->SBUF copy is fully contiguous.
                pv = ps.tile(
                    [128, H * (DH + 1)], F32, tag="sc", bufs=3, name=f"pv{st}"
                )
                for f in range(2):
                    nc.tensor.matmul(
                        pv[:],
                        vT_t[:, f, ts(st, 128)],
                        wv_t[:, f, :],
                        start=(f == 0),
                        stop=False,
                    )
                nc.tensor.matmul(
                    pv[:],
                    vrow1[0:1, ts(st, 128)],
                    wva_t[:],
                    start=False,
                    stop=True,
                )
                copy_to(ce, vp[:, st, :], pv[:])

            # attn[(m, t)] = exp(scoresT) for head pair t of quad m:
            # [k-chunk part, ct, r', q]
            attn = {}
            for m in range(2):
                for t in range(2):
                    attn[(m, t)] = sb.tile(
                        [128, NKT, 2, 512], BF16, tag="attn", bufs=4,
                        name=f"attn{m}{t}",
                    )

            def qk_fill(m, t, ct):
                slot = ps.tile(
                    [128, 2, 512], F32, tag="sc", bufs=3, name=f"sc{m}{t}{ct}"
                )
                for rr in range(2):
                    r = 2 * t + rr
                    nc.tensor.matmul(
                        slot[:, rr, :],
                        kp[ts(r, 32), m, ts(ct, 128)],
                        qp[ts(r, 32), m, :],
                        start=True,
                        stop=True,
                        tile_position=(32 * r, 0),
                    )
                return slot

            def qk_act(m, t, ct, slot, eng):
                dst = attn[(m, t)][:, ct, :, :]
                if eng == "A":
                    nc.scalar.activation(dst, slot[:], EXP)
                else:
                    nc.vector.tensor_scalar(
                        dst.bitcast(I16), slot[:], SCH_K, SCH_B, MULT, ADD
                    )

            po = {}  # (m, t) -> PSUM accumulator [128, 512]

            def pv_pair(m, t, ct):
                """PV for both heads of pair (m,t), k-chunk ct.  The two
                matmuls sit at col positions 0/64 so they run concurrently."""
                if ct == 0:
                    po[(m, t)] = ps.tile(
                        [128, 512], F32, tag="po", bufs=2, name=f"po{m}{t}"
                    )
                p = po[(m, t)]
                for rr in range(2):
                    h = 4 * m + 2 * t + rr
                    base = 64 * rr
                    nc.tensor.matmul(
                        p[base : base + DH + 1, :],
                        vp[:, ct, ts(h, DH + 1)],
                        attn[(m, t)][:, ct, rr, :],
                        start=(ct == 0),
                        stop=(ct == NKT - 1),
                        tile_position=(0, base),
                        skip_group_check=True,
                    )

            prod = {}

            def stage_c(m, t, bc=None):
                """normalize: prod = po * (1 / PE-broadcast(denominator rows)).

                dsb copies ride ACT; reciprocal + the single full-tile
                multiply ride DVE (tensor_tensor is DVE-only)."""
                p = po[(m, t)]
                dsb = sb.tile([128, 512], BF16, tag="dsb", bufs=2, name=f"dsb{m}{t}")
                if bc is None:
                    bc = ps.tile([128, 512], F32, tag="sc", bufs=3, name=f"bc{m}{t}")
                rsb = sb.tile([128, 512], F32, tag="rsb", bufs=2, name=f"rsb{m}{t}")
                prod[(m, t)] = sb.tile(
                    [128, 512], BF16, tag="prod", bufs=4, name=f"prod{m}{t}"
                )
                for base in (0, 64):
                    nc.scalar.copy(
                        dsb[base + DH : base + DH + 1, :],
                        p[base + DH : base + DH + 1, :],
                    )
                    # M=64 fills bc completely so the full-tile reciprocal
                    # below reads no stale slot bytes.
                    nc.tensor.matmul(
                        bc[base : base + 64, :],
                        ones_t[base + DH : base + DH + 1, :],
                        dsb[base + DH : base + DH + 1, :],
                        start=True,
                        stop=True,
                        tile_position=(base + DH, base),
                        skip_group_check=True,
                    )
                nc.vector.reciprocal_approx_fast(rsb[:], bc[:])
                # full-tile multiply: rows outside the head bands compute
                # garbage that nothing reads (pf matmuls take 0-31/64-95).
                nc.vector.tensor_mul(prod[(m, t)][:, :], p[:, :], rsb[:, :])

            # ================= schedule =================
            # startup: q-proj as soon as its (early) DMA lands, a dummy
            # batch to bridge the PE gap until kT arrives, then the k-proj
            # prefix for the first slots.  Both copies on DVE (idle then).
            proj_q(0, "D")
            keep_warm(6)
            proj_k(0, 0, "D")

            # work items carry a rough warm-PE cost (ns) so each slot gets
            # topped up with dummies to ~the exp drain cadence.
            def PV(m, t, ct):
                return (324, lambda: pv_pair(m, t, ct))

            def SC(m, t):
                return (546, lambda: stage_c(m, t))

            def PK(m, s4, ce):
                return (590, lambda: proj_k(m, s4, ce))

            def PQ(m, ce):
                return (590, lambda: proj_q(m, ce))

            def PVJ(st, ce):
                return (545, lambda: proj_v(st, ce))

            # Slot order INTERLEAVES the two head pairs of each quad:
            # slot(m, t, ct) = 32m + 2ct + t.  Consecutive slots are the
            # two pairs at the SAME k-chunk, so the two ring-replacement
            # fills issued together use disjoint PE row bands (rows
            # 0-63 for t=0, 64-127 for t=1) and stream 4-concurrently,
            # and PV work spreads evenly at 2 per slot pair instead of
            # bunching in the back phases.
            flat = []  # (m, t, ct)
            for m in range(2):
                for ct in range(NKT):
                    for t in range(2):
                        flat.append((m, t, ct))

            items = {j: [] for j in range(64)}
            # k/q projections: kT chunk s4 lands by ~slot 2-4; chunk ct
            # fills happen at slot 2ct so PK(m, s4) must complete ~8 slots
            # ahead of slot 8*s4 (m=0) / 32+8*s4 (m=1).
            items[0].append(PK(0, 1, "A"))
            items[4].append(PK(0, 2, "A"))
            items[8].append(PK(0, 3, "A"))
            items[12].append(PK(1, 0, "A"))
            items[14].append(PQ(1, "D"))
            items[16].append(PK(1, 1, "D"))
            items[20].append(PK(1, 2, "A"))
            items[24].append(PK(1, 3, "D"))
            # v projections: one per odd slot; vp[st] is consumed by PV at
            # slot 2st+6, giving the PSUM->SBUF copy 2 pairs of margin.
            _vce = ["D", "A"] * 8
            for st in range(NKT):
                items[2 * st + 1].append(PVJ(st, _vce[st]))
            # PV chases the acts with a 2-3 pair lag.  m=0 fits entirely
            # in-stream; m=1's last three chunks drain in the tail.
            for ct in range(NKT):
                for t in range(2):
                    items[2 * ct + 6 + t].append(PV(0, t, ct))
            for ct in range(13):
                for t in range(2):
                    items[38 + 2 * ct + t].append(PV(1, t, ct))
            # normalizations for quad 0 run right after its last PVs; the
            # SC(0,0) multiply must drain before PV(1,0,0) reuses its po
            # bank, so it leads slot 38's item list.
            items[38].insert(0, SC(0, 0))
            items[39].insert(1, SC(0, 1))

            slots = {}
            for j in range(2):
                slots[j] = qk_fill(*flat[j])
            for jp in range(32):
                j0, j1 = 2 * jp, 2 * jp + 1
                qk_act(*flat[j0], slots.pop(j0), ENG[j0])
                qk_act(*flat[j1], slots.pop(j1), ENG[j1])
                cost = 335  # fill pair
                for c, w in items[j0] + items[j1]:
                    w()
                    cost += c
                n_dmy = max(0, min(4, round((1560 - cost) / 215)))
                if jp < 8:
                    # the early pairs' items are DMA/copy-gated: their PE
                    # work stalls regardless of nominal cost, so keep a
                    # high dummy floor to saturate the HAM windows (the PE
                    # is DMA-paced here, so the padding is nearly free).
                    n_dmy = max(n_dmy, 4)
                keep_warm(n_dmy)
                for j in (j0 + 2, j1 + 2):
                    if j < len(flat):
                        slots[j] = qk_fill(*flat[j])

            # ---- tail.  PSUM tiles in dependency-safe ring order: bc11
            # first (so the last normalize is never gated on the final
            # projection), then the final-projection accumulators.  3/4 of
            # the output projection runs before the last normalize; only
            # (m1,t1)'s K=32 contribution is serialized after it.
            # drain: the last three k-chunks of quad 1 (their acts occupy
            # the final stream slots), then BOTH remaining normalizations.
            # The pf accumulators are allocated only after both stage_c's
            # so every ring-buffer reuse WARs an already-issued reader
            # (allocating them earlier deadlocks: the pf writes would be
            # ordered before the bc broadcast they transitively feed).
            for ct in (13, 14, 15):
                for t in range(2):
                    pv_pair(1, t, ct)
            stage_c(1, 0)
            stage_c(1, 1)

            out_sb = cst.tile([128, 2, QB], F32)
            pf = {}
            pf[(0, 0)] = ps.tile([128, QB], F32, tag="sc", bufs=3, name="pf00")
            pf[(0, 64)] = ps.tile([128, QB], F32, tag="sc", bufs=3, name="pf064")
            pf[(1, 0)] = ps.tile([128, QB], F32, tag="po", bufs=2, name="pf10")
            pf[(1, 64)] = ps.tile([128, QB], F32, tag="po", bufs=2, name="pf164")

            def pf_mms(idx, m, t):
                for o in range(2):
                    for base in (0, 64):
                        nc.tensor.matmul(
                            pf[(o, base)][:],
                            wo_t[base : base + DH, 2 * m + t, ts(o, 128)],
                            prod[(m, t)][base : base + DH, :],
                            start=(idx == 0),
                            stop=(idx == 3),
                            tile_position=(base, 0),
                            skip_group_check=True,
                        )

            for idx, (m, t) in enumerate([(0, 0), (0, 1), (1, 0), (1, 1)]):
                pf_mms(idx, m, t)
            for o in range(2):
                # bias add on ACT (Identity + per-partition bias), the
                # second accumulator add + out DMA on DVE/sync.
                nc.scalar.add(out_sb[:, o, :], pf[(o, 0)][:], bo_t[:, o, :])
                nc.vector.tensor_add(
                    out_sb[:, o, :], out_sb[:, o, :], pf[(o, 64)][:]
                )
                nc.sync.dma_start(outT[ts(o, 128), :], out_sb[:, o, :])

    nc.compile()
    return nc


_NC = None


def _get_nc():
    global _NC
    if _NC is None:
        _NC = _build_graph()
    return _NC


def prep_in_maps(query, key, value, Wq, Wk, Wv, Wo, bo):
    bf = ml_dtypes.bfloat16
    scale = np.float32(1.0 / np.sqrt(DH))

    wqT = np.ascontiguousarray((Wq.astype(np.float32) * scale).T).astype(bf)
    wkT = np.ascontiguousarray(Wk.T).astype(bf)
    # augmented WvT: [257 in-feats (last = ones row), 8 heads x 33 out-cols]
    wvT_a = np.zeros((D + 1, H * (DH + 1)), np.float32)
    wvt = Wv.T.astype(np.float32)  # [in 256, out 256]
    for h in range(H):
        wvT_a[:D, (DH + 1) * h : (DH + 1) * h + DH] = wvt[:, DH * h : DH * (h + 1)]
        wvT_a[D, (DH + 1) * h + DH] = 1.0
    wvT = np.ascontiguousarray(wvT_a).astype(bf)
    # permuted WoT: head h = 4m + 2t + rr lives at partition rows
    # 64*rr .. +32 of free-block 2m+t, matching PV output partitions.
    woP = np.zeros((128, 4, D), np.float32)
    woT = Wo.T.astype(np.float32)  # [hd, out]
    for h in range(H):
        m, r = h // 4, h % 4
        blk, base = 2 * m + r // 2, 64 * (r % 2)
        woP[base : base + DH, blk, :] = woT[DH * h : DH * (h + 1), :]
    woP = np.ascontiguousarray(woP.reshape(128, 4 * D)).astype(bf)
    bo_c = np.ascontiguousarray(bo.astype(np.float32).reshape(D, 1))

    kT_b = [np.ascontiguousarray(key[b].T).astype(bf) for b in range(B)]
    vT_b = [np.ascontiguousarray(value[b].T).astype(bf) for b in range(B)]

    in_maps = []
    for c in range(8):
        b, qb = c // 4, c % 4
        # one blob = wk | wq | qT so the startup-critical path is a single
        # DMA per f-half
        wkq = np.empty((D, 2 * D + QB), ml_dtypes.bfloat16)
        wkq[:, :D] = wkT
        wkq[:, D : 2 * D] = wqT
        wkq[:, 2 * D :] = query[b, qb * QB : (qb + 1) * QB, :].T.astype(bf)
        in_maps.append(
            {
                "wkq": np.ascontiguousarray(wkq),
                "kT": kT_b[b],
                "vT": vT_b[b],
                "wvT": wvT,
                "woP": woP,
                "bo": bo_c,
            }
        )
    return in_maps


def kernel(query, key, value, Wq, Wk, Wv, Wo, bo):
    nc = _get_nc()
    in_maps = prep_in_maps(query, key, value, Wq, Wk, Wv, Wo, bo)
    res = run_bass_kernel_spmd(nc, in_maps, core_ids=list(range(8)))

    out = np.empty((B, SEQ, D), np.float32)
    for c in range(8):
        b, qb = c // 4, c % 4
        out[b, qb * QB : (qb + 1) * QB, :] = res.results[c]["outT"].T
    return out


# revision 25
# speedup vs baseline: 1.0936x; 1.0782x over previous
"""Multi-head attention (B=2, S=2048, H=8, Dh=32, D=256) on 8 TRN2 NeuronCores.

Sharding: core c -> (batch b = c//4, query-block qb = c%4 of 512 rows).
Each core computes full attention + output projection for its 512 query rows;
no cross-core communication is needed.  Host does layout prep only
(transposes + bf16 casts); all FLOPs run on device.

Device-side layout (per core):
  - raw activations shipped transposed: qT [256f, 512q], kT/vT [256f, 2048s]
  - q/k projections produced directly transposed (head-dim on partitions,
    head h at partitions 32*(h%4) of free-block h//4) so QK^T runs as
    K=32 row-tiled matmuls; the two heads of a pair are issued
    back-to-back at row positions 32r so the PE runs them concurrently.
  - scores computed TRANSPOSED: scoresT[k, q]; no max subtraction
    (scores ~ N(0,1), |s| < 10 measured, exp overflow at 88).
  - v projected to natural layout augmented with a ones column per head
    ([128, 16, 8, 33]) so each PV matmul (M=33) also accumulates the
    softmax denominator as its last output row - no separate reduction.
  - normalization: reciprocal of the two denominator rows (partitions 32
    and 96) + K=1 matmul row-broadcast + one full-tile DVE multiply.
  - final projection: K=32 matmuls per head slice against a host-permuted
    WoT whose row bands match the PV output partition bases.

Schedule (final): the 64 exp tiles ([128, 2, 512] PSUM score slots) are
split between TWO engines running concurrently (alternating slots):
  - ScalarE (ACT): exact exp ACTIVATE, ~1.07us/tile (34 tiles).
  - VectorE (DVE): Schraudolph bit-trick exp in ONE tensor_scalar op:
      int16(round(score * 128/ln2 + (16256 - 7.5))) bitcast to bf16
    ~= exp(score) with ~2% element error, ~1.21us/tile (30 tiles).
    HW-verified: f32->int16 conversion rounds to nearest and saturates.
    The softmax denominator is accumulated from the SAME approximated
    weights (ones column), so the ratio cancels most of the error;
    measured end-to-end rel err ~1.3e-2 vs the 2e-2 gate.
Slot order interleaves the two head pairs of each quad (slot = 32m +
2ct + t) so the paired ring-replacement fills use disjoint PE row bands
and PV work spreads evenly.  Aux elementwise work (projection casts,
denominator row copies, tail bias adds) is distributed between ACT and
DVE to balance their streams; ACT Copy/Identity share Exp's activation
table set so interleaving costs no table switches.

The PE's HAM clock gate is the sharpest constraint: the PE runs 2.4 GHz
only after ~3.4us of near-gapless activity (free-running 4096-cycle
windows) and falls to 1.2 GHz after any window with idle slivers.  A
26-dummy burst during the startup DMA wait earns the lease before real
work begins, and every slot pair is topped up with small dep-free dummy
matmuls ([128,256], ~150ns) toward a ~1.56us pair cadence; removing
them (or letting any region run dependency-gated without padding) was
measured to drop the clock for the rest of the run and cost 15-30%.
PSUM budget: 2x[128,2,512] rotating score slots + 2x[128,512] PV
accumulators + tmp + dummy bank = 8 banks.
"""

import sys

sys.path.insert(0, "/opt/trn_rl_repo")

import numpy as np
import ml_dtypes

import concourse.bass as bass
import concourse.bacc as bacc
import concourse.mybir as mybir
from concourse.tile import TileContext
from concourse.bass import ts
from concourse.bass_utils import run_bass_kernel_spmd

BF16 = mybir.dt.bfloat16
F32 = mybir.dt.float32
I16 = mybir.dt.int16
EXP = mybir.ActivationFunctionType.Exp
MULT = mybir.AluOpType.mult
ADD = mybir.AluOpType.add

B, SEQ, D = 2, 2048, 256
H, DH = 8, 32
QB = 512  # query rows per core
NKT = SEQ // 128  # 16 k-chunk tiles (partition tiles of scoresT)

# Schraudolph exp in bf16-bit space: bf16_bits(exp(s)) ~= s*K + B_SCH
SCH_K = float(128.0 / np.log(2.0))
SCH_B = 16256.0 - 7.5  # bias tuned on end-to-end softmax error

# exp-engine per slot j (64 slots): alternate ACT/DVE; a couple of extra
# ACT slots rebalance for DVE's aux-heavier stream.
_EXTRA_ACT = (33, 49)
ENG = ["A" if (j % 2 == 0 or j in _EXTRA_ACT) else "D" for j in range(64)]


def _build_graph():
    nc = bacc.Bacc("TRN2", target_bir_lowering=False, debug=False)

    wkq = nc.declare_dram_parameter("wkq", [D, 2 * D + QB], BF16, isOutput=False)
    kT = nc.declare_dram_parameter("kT", [D, SEQ], BF16, isOutput=False)
    vT = nc.declare_dram_parameter("vT", [D, SEQ], BF16, isOutput=False)
    wvT = nc.declare_dram_parameter("wvT", [D + 1, H * (DH + 1)], BF16, isOutput=False)
    woP = nc.declare_dram_parameter("woP", [128, 4 * D], BF16, isOutput=False)
    bo = nc.declare_dram_parameter("bo", [D, 1], F32, isOutput=False)
    outT = nc.declare_dram_parameter("outT", [D, QB], F32, isOutput=True)

    with TileContext(nc) as tc:
        with (
            tc.tile_pool(name="cst", bufs=1) as cst,
            tc.tile_pool(name="sb", bufs=1) as sb,
            tc.tile_pool(name="ps", bufs=2, space="PSUM") as ps,
        ):
            # ---- inputs.  All transfers ride the sync queue, with the
            # host-concatenated wk|wq|qT blob first (startup critical path).
            wkq_t = cst.tile([128, 2, 2 * D + QB], BF16)
            wk_t = wkq_t[:, :, 0:D]
            wq_t = wkq_t[:, :, D : 2 * D]
            qT_t = wkq_t[:, :, 2 * D :]
            kT_t = cst.tile([128, 2, SEQ], BF16)
            wv_t = cst.tile([128, 2, H * (DH + 1)], BF16)
            wva_t = cst.tile([1, H * (DH + 1)], BF16)
            vT_t = cst.tile([128, 2, SEQ], BF16)
            wo_t = cst.tile([128, 4, D], BF16)
            bo_t = cst.tile([128, 2, 1], F32)

            # ACT warm-up + exp table load before the stream starts.
            warm = cst.tile([1, 1], F32)
            nc.vector.memset(warm[:], 0.0)
            nc.scalar.activation(warm[:], warm[:], EXP)

            for f in range(2):
                nc.sync.dma_start(wkq_t[:, f, :], wkq[ts(f, 128), :])
            for f in range(2):
                nc.sync.dma_start(kT_t[:, f, ts(0, 512)], kT[ts(f, 128), ts(0, 512)])
            for f in range(2):
                nc.sync.dma_start(wv_t[:, f, :], wvT[ts(f, 128), :])
            nc.sync.dma_start(wva_t[:], wvT[D : D + 1, :])
            # early vT chunk: the interleaved schedule starts PV at slot ~6,
            # so the first v-projections need vT right after the k prefix.
            for f in range(2):
                nc.sync.dma_start(vT_t[:, f, ts(0, 512)], vT[ts(f, 128), ts(0, 512)])
            for f in range(2):
                nc.sync.dma_start(
                    kT_t[:, f, 512:1536], kT[ts(f, 128), 512:1536]
                )
            for f in range(2):
                nc.sync.dma_start(vT_t[:, f, 512:], vT[ts(f, 128), 512:])
            for f in range(2):
                nc.sync.dma_start(
                    kT_t[:, f, 1536:], kT[ts(f, 128), 1536:]
                )
            nc.sync.dma_start(wo_t[:, :, :], woP.rearrange("p (b d) -> p b d", b=4))
            for o in range(2):
                nc.sync.dma_start(bo_t[:, o, :], bo[ts(o, 128), :])

            # ---- SBUF destinations for projections.
            qp = cst.tile([128, 2, QB], BF16)  # q_projT
            kp = cst.tile([128, 2, SEQ], BF16)  # k_projT
            vp = cst.tile([128, NKT, H * (DH + 1)], BF16)  # v_proj + ones cols
            ones_t = cst.tile([128, 64], BF16)
            nc.vector.memset(ones_t[:], 1.0)
            vrow1 = cst.tile([1, SEQ], BF16)

            # ---- PE keep-warm machinery (HAM clock gate: the PE drops to
            # 1.2 GHz unless near-gaplessly busy; dep-free dummy matmuls
            # pepper the stream).  K=128 rows with an M=32 output keeps the
            # array visibly busy at a quarter of full PSUM write traffic.
            dmy = cst.tile([128, 256], BF16)
            nc.vector.memset(dmy[:], 0.0)
            nc.vector.memset(vrow1[:], 1.0)
            pdmy = ps.tile([128, 512], F32, tag="dmy", bufs=1, name="pdmy")

            def keep_warm(n):
                for _ in range(n):
                    nc.tensor.matmul(
                        pdmy[:32, 0:256],
                        ones_t[:, 0:32],
                        dmy[:, :],
                        start=True,
                        stop=True,
                    )

            # Earn the HAM warm lease during the startup DMA wait: the clock
            # only rises to 2.4 GHz after a sustained gapless stretch, so a
            # dense dummy burst here makes the real stream start warm.
            keep_warm(30)

            def copy_to(ce, dst, src):
                """PSUM f32 -> SBUF bf16 cast on the chosen engine."""
                if ce == "A":
                    nc.scalar.copy(dst, src)
                else:
                    nc.vector.tensor_copy(dst, src)

            # Transient projection PSUM pieces get a dedicated 1-slot ring
            # ("tmp"); "po" is reserved for the long-lived PV accumulators,
            # "sc" for the exp score slots.
            def proj_k(m, s4, ce):
                pk = ps.tile([128, 512], F32, tag="tmp", bufs=1, name=f"pk{m}{s4}")
                for f in range(2):
                    nc.tensor.matmul(
                        pk[:],
                        wk_t[:, f, ts(m, 128)],
                        kT_t[:, f, ts(s4, 512)],
                        start=(f == 0),
                        stop=(f == 1),
                    )
                copy_to(ce, kp[:, m, ts(s4, 512)], pk[:])

            def proj_q(m, ce):
                pq = ps.tile([128, QB], F32, tag="tmp", bufs=1, name=f"pq{m}")
                for f in range(2):
                    nc.tensor.matmul(
                        pq[:],
                        wq_t[:, f, ts(m, 128)],
                        qT_t[:, f, :],
                        start=(f == 0),
                        stop=(f == 1),
                    )
                copy_to(ce, qp[:, m, :], pq[:])

            def proj_v(st, ce):
                # third K=1 matmul of the host-side ones row against the
                # augmented Wv row produces the per-head ones columns, so
                # the PSUM->SBUF copy is fully contiguous.
                pv = ps.tile(
                    [128, H * (DH + 1)], F32, tag="tmp", bufs=1, name=f"pv{st}"
                )
                for f in range(2):
                    nc.tensor.matmul(
                        pv[:],
                        vT_t[:, f, ts(st, 128)],
                        wv_t[:, f, :],
                        start=(f == 0),
                        stop=False,
                    )
                nc.tensor.matmul(
                    pv[:],
                    vrow1[0:1, ts(st, 128)],
                    wva_t[:],
                    start=False,
                    stop=True,
                )
                copy_to(ce, vp[:, st, :], pv[:])

            # attn[(m, t)] = exp(scoresT) for head pair t of quad m:
            # [k-chunk part, ct, r', q]
            attn = {}
            for m in range(2):
                for t in range(2):
                    attn[(m, t)] = sb.tile(
                        [128, NKT, 2, 512], BF16, tag="attn", bufs=4,
                        name=f"attn{m}{t}",
                    )

            def qk_fill(m, t, ct):
                slot = ps.tile(
                    [128, 2, 512], F32, tag="sc", bufs=2, name=f"sc{m}{t}{ct}"
                )
                for rr in range(2):
                    r = 2 * t + rr
                    nc.tensor.matmul(
                        slot[:, rr, :],
                        kp[ts(r, 32), m, ts(ct, 128)],
                        qp[ts(r, 32), m, :],
                        start=True,
                        stop=True,
                        tile_position=(32 * r, 0),
                    )
                return slot

            def qk_act(m, t, ct, slot, eng):
                dst = attn[(m, t)][:, ct, :, :]
                if eng == "A":
                    nc.scalar.activation(dst, slot[:], EXP)
                else:
                    nc.vector.tensor_scalar(
                        dst.bitcast(I16), slot[:], SCH_K, SCH_B, MULT, ADD
                    )

            po = {}  # (m, t) -> PSUM accumulator [128, 512]

            def pv_pair(m, t, ct):
                """PV for both heads of pair (m,t), k-chunk ct.  The two
                matmuls sit at col positions 0/64 so they run concurrently."""
                if ct == 0:
                    po[(m, t)] = ps.tile(
                        [128, 512], F32, tag="po", bufs=2, name=f"po{m}{t}"
                    )
                p = po[(m, t)]
                for rr in range(2):
                    h = 4 * m + 2 * t + rr
                    base = 64 * rr
                    nc.tensor.matmul(
                        p[base : base + DH + 1, :],
                        vp[:, ct, ts(h, DH + 1)],
                        attn[(m, t)][:, ct, rr, :],
                        start=(ct == 0),
                        stop=(ct == NKT - 1),
                        tile_position=(0, base),
                        skip_group_check=True,
                    )

            prod = {}

            def stage_c(m, t, bc=None):
                """normalize: prod = po * (1 / PE-broadcast(denominator rows)).

                dsb copies ride ACT; reciprocal + the single full-tile
                multiply ride DVE (tensor_tensor is DVE-only)."""
                p = po[(m, t)]
                dsb = sb.tile([128, 512], BF16, tag="dsb", bufs=2, name=f"dsb{m}{t}")
                if bc is None:
                    bc = ps.tile([128, 512], F32, tag="tmp", bufs=1, name=f"bc{m}{t}")
                rsb = sb.tile([128, 512], F32, tag="rsb", bufs=2, name=f"rsb{m}{t}")
                prod[(m, t)] = sb.tile(
                    [128, 512], BF16, tag="prod", bufs=4, name=f"prod{m}{t}"
                )
                for base in (0, 64):
                    nc.scalar.copy(
                        dsb[base + DH : base + DH + 1, :],
                        p[base + DH : base + DH + 1, :],
                    )
                    # M=64 fills bc completely so the full-tile reciprocal
                    # below reads no stale slot bytes.
                    nc.tensor.matmul(
                        bc[base : base + 64, :],
                        ones_t[base + DH : base + DH + 1, :],
                        dsb[base + DH : base + DH + 1, :],
                        start=True,
                        stop=True,
                        tile_position=(base + DH, base),
                        skip_group_check=True,
                    )
                nc.vector.reciprocal_approx_fast(rsb[:], bc[:])
                # full-tile multiply: rows outside the head bands compute
                # garbage that nothing reads (pf matmuls take 0-31/64-95).
                nc.vector.tensor_mul(prod[(m, t)][:, :], p[:, :], rsb[:, :])

            # ================= schedule =================
            # startup: q-proj as soon as its (early) DMA lands, a dummy
            # batch to bridge the PE gap until kT arrives, then the k-proj
            # prefix for the first slots.  Both copies on DVE (idle then).
            proj_q(0, "D")
            keep_warm(6)
            proj_k(0, 0, "D")

            # work items carry a rough warm-PE cost (ns) so each slot gets
            # topped up with dummies to ~the exp drain cadence.
            def PV(m, t, ct):
                return (324, lambda: pv_pair(m, t, ct))

            def SC(m, t):
                return (546, lambda: stage_c(m, t))

            def PK(m, s4, ce):
                return (590, lambda: proj_k(m, s4, ce))

            def PQ(m, ce):
                return (590, lambda: proj_q(m, ce))

            def PVJ(st, ce):
                return (545, lambda: proj_v(st, ce))

            # Slot order INTERLEAVES the two head pairs of each quad:
            # slot(m, t, ct) = 32m + 2ct + t.  Consecutive slots are the
            # two pairs at the SAME k-chunk, so the two ring-replacement
            # fills issued together use disjoint PE row bands (rows
            # 0-63 for t=0, 64-127 for t=1) and stream 4-concurrently,
            # and PV work spreads evenly at 2 per slot pair instead of
            # bunching in the back phases.
            flat = []  # (m, t, ct)
            for m in range(2):
                for ct in range(NKT):
                    for t in range(2):
                        flat.append((m, t, ct))

            items = {j: [] for j in range(64)}
            # k/q projections: kT chunk s4 lands by ~slot 2-4; chunk ct
            # fills happen at slot 2ct so PK(m, s4) must complete ~8 slots
            # ahead of slot 8*s4 (m=0) / 32+8*s4 (m=1).
            items[0].append(PK(0, 1, "A"))
            items[4].append(PK(0, 2, "A"))
            items[8].append(PK(0, 3, "A"))
            items[12].append(PK(1, 0, "A"))
            items[14].append(PQ(1, "D"))
            items[16].append(PK(1, 1, "D"))
            items[20].append(PK(1, 2, "A"))
            items[24].append(PK(1, 3, "D"))
            # v projections: one per odd slot; vp[st] is consumed by PV at
            # slot 2st+6, giving the PSUM->SBUF copy 2 pairs of margin.
            _vce = ["D", "A"] * 8
            for st in range(NKT):
                items[2 * st + 1].append(PVJ(st, _vce[st]))
            # PV chases the acts with a 2-3 pair lag.  m=0 fits entirely
            # in-stream; m=1's last three chunks drain in the tail.
            for ct in range(NKT):
                for t in range(2):
                    items[2 * ct + 6 + t].append(PV(0, t, ct))
            for ct in range(13):
                for t in range(2):
                    items[38 + 2 * ct + t].append(PV(1, t, ct))
            # normalizations for quad 0 run right after its last PVs; the
            # SC(0,0) multiply must drain before PV(1,0,0) reuses its po
            # bank, so it leads slot 38's item list.
            items[38].insert(0, SC(0, 0))
            items[39].insert(1, SC(0, 1))

            slots = {}
            for j in range(2):
                slots[j] = qk_fill(*flat[j])
            for jp in range(32):
                j0, j1 = 2 * jp, 2 * jp + 1
                qk_act(*flat[j0], slots.pop(j0), ENG[j0])
                qk_act(*flat[j1], slots.pop(j1), ENG[j1])
                cost = 335  # fill pair
                for c, w in items[j0] + items[j1]:
                    w()
                    cost += c
                n_dmy = max(0, min(4, round((1560 - cost) / 215)))
                if jp < 8:
                    # the early pairs' items are DMA/copy-gated: their PE
                    # work stalls regardless of nominal cost, so keep a
                    # dummy floor to protect the clock lease.
                    n_dmy = max(n_dmy, 3)
                keep_warm(n_dmy)
                for j in (j0 + 2, j1 + 2):
                    if j < len(flat):
                        slots[j] = qk_fill(*flat[j])

            # ---- tail.  PSUM tiles in dependency-safe ring order: bc11
            # first (so the last normalize is never gated on the final
            # projection), then the final-projection accumulators.  3/4 of
            # the output projection runs before the last normalize; only
            # (m1,t1)'s K=32 contribution is serialized after it.
            # drain: the last three k-chunks of quad 1 (their acts occupy
            # the final stream slots), then the remaining normalizations.
            for ct in (13, 14, 15):
                for t in range(2):
                    pv_pair(1, t, ct)
            stage_c(1, 0)

            bc11 = ps.tile([128, 512], F32, tag="tmp", bufs=1, name="bc11")
            out_sb = cst.tile([128, 2, QB], F32)
            pf = {}
            pf[(0, 0)] = ps.tile([128, QB], F32, tag="sc", bufs=2, name="pf00")
            pf[(0, 64)] = ps.tile([128, QB], F32, tag="sc", bufs=2, name="pf064")
            pf[(1, 0)] = ps.tile([128, QB], F32, tag="po", bufs=2, name="pf10")
            pf[(1, 64)] = ps.tile([128, QB], F32, tag="dmy", bufs=1, name="pf164")

            def pf_mms(idx, m, t):
                for o in range(2):
                    for base in (0, 64):
                        nc.tensor.matmul(
                            pf[(o, base)][:],
                            wo_t[base : base + DH, 2 * m + t, ts(o, 128)],
                            prod[(m, t)][base : base + DH, :],
                            start=(idx == 0),
                            stop=(idx == 3),
                            tile_position=(base, 0),
                            skip_group_check=True,
                        )

            for idx, (m, t) in enumerate([(0, 0), (0, 1), (1, 0)]):
                pf_mms(idx, m, t)
            stage_c(1, 1, bc=bc11)
            pf_mms(3, 1, 1)
            for o in range(2):
                # bias add on ACT (Identity + per-partition bias), the
                # second accumulator add + out DMA on DVE/sync.
                nc.scalar.add(out_sb[:, o, :], pf[(o, 0)][:], bo_t[:, o, :])
                nc.vector.tensor_add(
                    out_sb[:, o, :], out_sb[:, o, :], pf[(o, 64)][:]
                )
                nc.sync.dma_start(outT[ts(o, 128), :], out_sb[:, o, :])

    nc.compile()
    return nc


_NC = None


def _get_nc():
    global _NC
    if _NC is None:
        _NC = _build_graph()
    return _NC


def prep_in_maps(query, key, value, Wq, Wk, Wv, Wo, bo):
    bf = ml_dtypes.bfloat16
    scale = np.float32(1.0 / np.sqrt(DH))

    wqT = np.ascontiguousarray((Wq.astype(np.float32) * scale).T).astype(bf)
    wkT = np.ascontiguousarray(Wk.T).astype(bf)
    # augmented WvT: [257 in-feats (last = ones row), 8 heads x 33 out-cols]
    wvT_a = np.zeros((D + 1, H * (DH + 1)), np.float32)
    wvt = Wv.T.astype(np.float32)  # [in 256, out 256]
    for h in range(H):
        wvT_a[:D, (DH + 1) * h : (DH + 1) * h + DH] = wvt[:, DH * h : DH * (h + 1)]
        wvT_a[D, (DH + 1) * h + DH] = 1.0
    wvT = np.ascontiguousarray(wvT_a).astype(bf)
    # permuted WoT: head h = 4m + 2t + rr lives at partition rows
    # 64*rr .. +32 of free-block 2m+t, matching PV output partitions.
    woP = np.zeros((128, 4, D), np.float32)
    woT = Wo.T.astype(np.float32)  # [hd, out]
    for h in range(H):
        m, r = h // 4, h % 4
        blk, base = 2 * m + r // 2, 64 * (r % 2)
        woP[base : base + DH, blk, :] = woT[DH * h : DH * (h + 1), :]
    woP = np.ascontiguousarray(woP.reshape(128, 4 * D)).astype(bf)
    bo_c = np.ascontiguousarray(bo.astype(np.float32).reshape(D, 1))

    kT_b = [np.ascontiguousarray(key[b].T).astype(bf) for b in range(B)]
    vT_b = [np.ascontiguousarray(value[b].T).astype(bf) for b in range(B)]

    in_maps = []
    for c in range(8):
        b, qb = c // 4, c % 4
        # one blob = wk | wq | qT so the startup-critical path is a single
        # DMA per f-half
        wkq = np.empty((D, 2 * D + QB), ml_dtypes.bfloat16)
        wkq[:, :D] = wkT
        wkq[:, D : 2 * D] = wqT
        wkq[:, 2 * D :] = query[b, qb * QB : (qb + 1) * QB, :].T.astype(bf)
        in_maps.append(
            {
                "wkq": np.ascontiguousarray(wkq),
                "kT": kT_b[b],
                "vT": vT_b[b],
                "wvT": wvT,
                "woP": woP,
                "bo": bo_c,
            }
        )
    return in_maps


def kernel(query, key, value, Wq, Wk, Wv, Wo, bo):
    nc = _get_nc()
    in_maps = prep_in_maps(query, key, value, Wq, Wk, Wv, Wo, bo)
    res = run_bass_kernel_spmd(nc, in_maps, core_ids=list(range(8)))

    out = np.empty((B, SEQ, D), np.float32)
    for c in range(8):
        b, qb = c // 4, c % 4
        out[b, qb * QB : (qb + 1) * QB, :] = res.results[c]["outT"].T
    return out
